# revision 15
# baseline (speedup 1.0000x reference)
"""Trainium2 Bass kernel for CoocOpModel.

out[b,s,z] = sum_{i,j} func[b,s,i] * cooc[i,j,z] * arg[b,s,j]
  with func = func_and_arg[..., :128], arg = func_and_arg[..., 128:]

Shapes (hardcoded): func_and_arg [4,1024,256] f32, cooccurrences [128,128,128] f32,
out [4,1024,128] f32.  D = 128, tokens T = 4096.

Strategy: data-parallel over tokens across 8 cores (512 tokens/core);
cooccurrence tensor replicated per core (fp16).

Per-core math, with t = local token index (512), i/j/z in [0,128):
  out_T[z, t] = sum_i  C_i^T @ G_i        (accumulated in one PSUM bank)
  C_i[j, z]   = cooc[i, j, z]             (stationary operand, fp16)
  G_i[j, t]   = arg_T[j, t] * func_T[i, t]  (moving operand, fp16)

For each i-group we need f_exp[j, (k,t)] = func_T[i0+k, t] replicated
across the 128 j-partitions; a DVE tensor-tensor multiply then builds G
and the per-i matmuls accumulate into PSUM.

The replication (D*D*T_core*2B = 16 MB/core) saturates the per-core DMA
fabric (~350 GB/s) if done purely as broadcast-DMA from DRAM, so it is
split across two otherwise-idle resources:
  - 'd' groups: broadcast-DMA from DRAM (partition-step-0 source AP) on
    the two hardware-DGE queues (sync + scalar)
  - 'g' groups: GpSimd InstPartitionBroadcast replicates f rows (staged
    once on SBUF partition 0) to all 128 partitions, SBUF->SBUF in bf16,
    zero DMA-fabric bytes and no PE/ACT involvement.
cooc tiles ride DVE-issued hardware-DGE DMAs (the DVE sequencer is free
while its engine runs the TT multiplies), two positions ahead, so the fx
queues never park a critical broadcast behind a cooc transfer and fx
flow-control stalls never block cooc delivery.
"""

import sys

sys.path.insert(0, "/opt/trn_rl_repo")

import numpy as np
from contextlib import ExitStack

import concourse.bass as bass
import concourse.tile as tile
from concourse import bacc, mybir
from concourse.bass_utils import run_bass_kernel_spmd

BF16 = mybir.dt.float16
F32 = mybir.dt.float32
NP_BF16 = np.float16

N_CORES = 8
D = 128
T_TOTAL = 4096
T_CORE = T_TOTAL // N_CORES  # 512

# Execution schedule: ('d'|'g', n_rows); sum = 128.
SCHED = (
    [("d", 4), ("d", 4)]
    + [("g", 8), ("d", 8)] * 7
    + [("d", 4), ("d", 4)]
)
assert sum(r for _, r in SCHED) == D
G_ROWS = sum(r for k, r in SCHED if k == "g")
I_G0 = D - G_ROWS  # g-groups own the top i-range [I_G0, 128)

_NC_CACHE = None


def _build():
    nc = bacc.Bacc("TRN2", target_bir_lowering=False, debug=False, num_devices=N_CORES)

    f_t = nc.dram_tensor("f_t", [D, T_CORE], BF16, kind="ExternalInput").ap()
    a_t = nc.dram_tensor("a_t", [D, T_CORE], BF16, kind="ExternalInput").ap()
    # c2[j, i*128 + z] = cooc[i, j, z]
    c2 = nc.dram_tensor("c2", [D, D * D], BF16, kind="ExternalInput").ap()
    out_t = nc.dram_tensor("out_t", [D, T_CORE], F32, kind="ExternalOutput").ap()

    with tile.TileContext(nc) as tc:
        with ExitStack() as ctx:
            const_pool = ctx.enter_context(tc.tile_pool(name="const", bufs=1))
            fexp_pool = ctx.enter_context(tc.tile_pool(name="fexp", bufs=6))
            fxg_pool = ctx.enter_context(tc.tile_pool(name="fxg", bufs=3))
            g_pool = ctx.enter_context(tc.tile_pool(name="g", bufs=3))
            out_pool = ctx.enter_context(tc.tile_pool(name="out", bufs=1))
            psum_pool = ctx.enter_context(
                tc.tile_pool(name="psum", bufs=1, space="PSUM")
            )

            a_sb = const_pool.tile([D, T_CORE], BF16, tag="a")
            nc.sync.dma_start(a_sb[:], a_t[:, :])
            a_ap = a_sb[:]

            # f rows for the g-groups, staged on SBUF partition 0
            f_pe = const_pool.tile([1, G_ROWS * T_CORE], BF16, tag="fpe")
            f_pe_src = bass.AP(
                f_t.tensor, I_G0 * T_CORE, [[0, 1], [1, G_ROWS * T_CORE]]
            )
            nc.scalar.dma_start(f_pe[:], f_pe_src)

            ps = psum_pool.tile([D, T_CORE], F32)

            meta = []
            d_i0, g_i0 = 0, I_G0
            for kind, sz in SCHED:
                if kind == "d":
                    meta.append((kind, sz, d_i0))
                    d_i0 += sz
                else:
                    meta.append((kind, sz, g_i0))
                    g_i0 += sz
            n_pos = len(meta)

            # cooc tiles: first three up-front (sync/scalar/vector), the rest
            # issued from the DVE sequencer three positions ahead.
            c_tiles = {}

            def issue_c(p, eng):
                kind, sz, i0 = meta[p]
                c_sb = const_pool.tile([D, sz * D], BF16, tag=f"c{p}")
                eng.dma_start(c_sb[:], c2[:, i0 * D : (i0 + sz) * D])
                c_tiles[p] = c_sb

            issue_c(0, nc.sync)
            issue_c(1, nc.scalar)
            issue_c(2, nc.sync)

            # g-group broadcasts, emitted eagerly (Pool is otherwise idle;
            # fxg pool depth throttles it naturally)
            fxg_tiles = {}

            def emit_g_broadcast(p):
                kind, sz, i0 = meta[p]
                fx = fxg_pool.tile([D, sz * T_CORE], BF16, tag="fxg")
                off = (i0 - I_G0) * T_CORE
                nc.gpsimd.partition_broadcast(
                    fx[:], f_pe[:, off : off + sz * T_CORE]
                )
                fxg_tiles[p] = fx

            first = True
            dq = 0
            for p in range(n_pos):
                kind, sz, i0 = meta[p]

                # keep g broadcasts two positions ahead
                for pp in range(p, min(p + 3, n_pos)):
                    if meta[pp][0] == "g" and pp not in fxg_tiles:
                        emit_g_broadcast(pp)

                if kind == "d":
                    fx = fexp_pool.tile([D, sz * T_CORE], BF16, tag="fxd")
                    if p == 0:
                        half = sz // 2
                        src_a = bass.AP(
                            f_t.tensor,
                            i0 * T_CORE,
                            [[0, D], [T_CORE, half], [1, T_CORE]],
                        )
                        src_b = bass.AP(
                            f_t.tensor,
                            (i0 + half) * T_CORE,
                            [[0, D], [T_CORE, half], [1, T_CORE]],
                        )
                        nc.scalar.dma_start(fx[:, : half * T_CORE], src_a)
                        nc.sync.dma_start(fx[:, half * T_CORE :], src_b)
                    else:
                        src = bass.AP(
                            f_t.tensor,
                            i0 * T_CORE,
                            [[0, D], [T_CORE, sz], [1, T_CORE]],
                        )
                        eng = nc.sync if dq % 2 == 0 else nc.scalar
                        eng.dma_start(fx[:], src)
                    dq += 1
                else:
                    fx = fxg_tiles[p]

                a_view = bass.AP(
                    a_ap.tensor, a_ap.offset, [a_ap.ap[0], [0, sz], [1, T_CORE]]
                )
                gt = g_pool.tile([D, sz * T_CORE], BF16, tag="g")
                nc.vector.tensor_mul(gt[:], a_view, fx[:])

                # prefetch a later position's cooc tile on the queue the
                # current fx did not use
                if p + 3 < n_pos:
                    issue_c(p + 3, nc.scalar if dq % 2 == 1 else nc.sync)

                c_sb = c_tiles.pop(p)
                for k in range(sz):
                    last = (p == n_pos - 1) and (k == sz - 1)
                    nc.tensor.matmul(
                        ps[:],
                        c_sb[:, k * D : (k + 1) * D],
                        gt[:, k * T_CORE : (k + 1) * T_CORE],
                        start=first,
                        stop=last,
                    )
                    first = False

            # drain: split output halves across two copy engines + queues
            o_sb = out_pool.tile([D, T_CORE], F32, tag="o")
            h = T_CORE // 2
            nc.vector.tensor_copy(o_sb[:, :h], ps[:, :h])
            nc.sync.dma_start(out_t[:, :h], o_sb[:, :h])
            nc.scalar.copy(o_sb[:, h:], ps[:, h:])
            nc.scalar.dma_start(out_t[:, h:], o_sb[:, h:])

    nc.compile()
    return nc


def _get_nc():
    global _NC_CACHE
    if _NC_CACHE is None:
        _NC_CACHE = _build()
    return _NC_CACHE


def _prep_in_maps(func_and_arg, cooccurrences):
    fa = np.asarray(func_and_arg, dtype=np.float32).reshape(T_TOTAL, 2 * D)
    c2 = (
        np.ascontiguousarray(
            np.asarray(cooccurrences, dtype=np.float32).transpose(1, 0, 2)
        )
        .reshape(D, D * D)
        .astype(NP_BF16)
    )
    in_maps = []
    for c in range(N_CORES):
        s = fa[c * T_CORE : (c + 1) * T_CORE]  # [512, 256]
        f_tc = np.ascontiguousarray(s[:, :D].T).astype(NP_BF16)  # [128 i, 512 t]
        a_tc = np.ascontiguousarray(s[:, D:].T).astype(NP_BF16)  # [128 j, 512 t]
        in_maps.append({"f_t": f_tc, "a_t": a_tc, "c2": c2})
    return in_maps


def kernel(func_and_arg: np.ndarray, cooccurrences: np.ndarray) -> np.ndarray:
    assert func_and_arg.shape == (4, 1024, 2 * D)
    assert cooccurrences.shape == (D, D, D)

    in_maps = _prep_in_maps(func_and_arg, cooccurrences)
    nc = _get_nc()
    res = run_bass_kernel_spmd(nc, in_maps, core_ids=list(range(N_CORES)))

    # out_t per core: [z=128, t=512] -> [t, z]; concat over cores -> [4096, 128]
    outs = [res.results[c]["out_t"].T for c in range(N_CORES)]
    out = np.concatenate(outs, axis=0).reshape(4, 1024, D).astype(np.float32)
    return out


# revision 16
# speedup vs baseline: 1.3309x; 1.3309x over previous
"""Trainium2 Bass kernel for CoocOpModel.

out[b,s,z] = sum_{i,j} func[b,s,i] * cooc[i,j,z] * arg[b,s,j]
  with func = func_and_arg[..., :128], arg = func_and_arg[..., 128:]

Shapes (hardcoded): func_and_arg [4,1024,256] f32, cooccurrences [128,128,128] f32,
out [4,1024,128] f32.  D = 128, tokens T = 4096.

Strategy: data-parallel over tokens across 8 cores (512 tokens/core);
cooccurrence tensor replicated per core (fp16).

Per-core math, with t = local token index (512), i/j/z in [0,128):
  out_T[z, t] = sum_i  C_i^T @ G_i        (accumulated in one PSUM bank)
  C_i[j, z]   = cooc[i, j, z]             (stationary operand, fp16)
  G_i[j, t]   = arg_T[j, t] * func_T[i, t]  (moving operand, fp16)

i's are processed in groups of GRP=8:
  - one broadcast-DMA materializes f_exp_g[j, (k,t)] = func_T[8g+k, t]
    (replicated across the 128 j-partitions; DRAM-source AP with
    partition-step 0 — SBUF sources reject step-0 partition dims)
  - one DVE tensor-tensor multiply builds G for the whole group, re-reading
    arg_T per k through a free-dim step-0 AP (no materialized a_rep)
  - 8 accumulating matmuls consume it (stationary = per-group cooc tile)
"""

import sys

sys.path.insert(0, "/opt/trn_rl_repo")

import numpy as np
from contextlib import ExitStack

import concourse.bass as bass
import concourse.tile as tile
from concourse import bacc, mybir
from concourse.bass_utils import run_bass_kernel_spmd

BF16 = mybir.dt.float16
F32 = mybir.dt.float32
NP_BF16 = np.float16

N_CORES = 8
D = 128
T_TOTAL = 4096
T_CORE = T_TOTAL // N_CORES  # 512
GRP = 8
N_GRP = D // GRP

# schedule knobs
FEXP_BUFS = 3
SIZES = [GRP] * N_GRP
SPLIT_OUT = False

_NC_CACHE = None


def _build():
    nc = bacc.Bacc("TRN2", target_bir_lowering=False, debug=False, num_devices=N_CORES)

    f_t = nc.dram_tensor("f_t", [D, T_CORE], BF16, kind="ExternalInput").ap()
    a_t = nc.dram_tensor("a_t", [D, T_CORE], BF16, kind="ExternalInput").ap()
    # c2[j, i*128 + z] = cooc[i, j, z]
    c2 = nc.dram_tensor("c2", [D, D * D], BF16, kind="ExternalInput").ap()
    out_t = nc.dram_tensor("out_t", [D, T_CORE], F32, kind="ExternalOutput").ap()

    with tile.TileContext(nc) as tc:
        with ExitStack() as ctx:
            const_pool = ctx.enter_context(tc.tile_pool(name="const", bufs=1))
            fexp_pool = ctx.enter_context(tc.tile_pool(name="fexp", bufs=FEXP_BUFS))
            g_pool = ctx.enter_context(tc.tile_pool(name="g", bufs=3))
            out_pool = ctx.enter_context(tc.tile_pool(name="out", bufs=1))
            psum_pool = ctx.enter_context(
                tc.tile_pool(name="psum", bufs=1, space="PSUM")
            )

            # arg_T in SBUF; the TT re-reads it per k via a free-step-0 AP.
            a_sb = const_pool.tile([D, T_CORE], BF16, tag="a")
            nc.sync.dma_start(a_sb[:], a_t[:, :])
            a_ap = a_sb[:]

            sizes = SIZES
            assert sum(sizes) == D

            ps = psum_pool.tile([D, T_CORE], F32)
            i0 = 0
            for g, sz in enumerate(sizes):
                # f_exp[j, (k, t)] = func_T[i0+k, t], replicated over j.
                f_exp = fexp_pool.tile([D, sz * T_CORE], BF16, tag="fexp")
                if g == 0:
                    half = sz // 2
                    f_src_a = bass.AP(
                        f_t.tensor, i0 * T_CORE, [[0, D], [T_CORE, half], [1, T_CORE]]
                    )
                    f_src_b = bass.AP(
                        f_t.tensor,
                        (i0 + half) * T_CORE,
                        [[0, D], [T_CORE, half], [1, T_CORE]],
                    )
                    nc.scalar.dma_start(f_exp[:, : half * T_CORE], f_src_a)
                    nc.sync.dma_start(f_exp[:, half * T_CORE :], f_src_b)
                else:
                    f_src = bass.AP(
                        f_t.tensor,
                        i0 * T_CORE,
                        [[0, D], [T_CORE, sz], [1, T_CORE]],
                    )
                    eng = nc.sync if g % 2 == 0 else nc.scalar
                    eng.dma_start(f_exp[:], f_src)

                # per-group cooc tile: c_sb[j, (k, z)] = cooc[i0+k, j, z]
                c_sb = const_pool.tile([D, sz * D], BF16, tag=f"c{g}")
                eng = nc.scalar if g % 2 == 0 else nc.sync
                eng.dma_start(c_sb[:], c2[:, i0 * D : (i0 + sz) * D])

                a_view = bass.AP(
                    a_ap.tensor, a_ap.offset, [a_ap.ap[0], [0, sz], [1, T_CORE]]
                )
                gt = g_pool.tile([D, sz * T_CORE], BF16, tag="g")
                if g == len(sizes) - 1 and sz >= 2:
                    # split the last multiply so its first matmuls overlap
                    # the second half — trims the serial tail
                    h = sz // 2
                    a_half = bass.AP(
                        a_ap.tensor, a_ap.offset, [a_ap.ap[0], [0, h], [1, T_CORE]]
                    )
                    nc.vector.tensor_mul(
                        gt[:, : h * T_CORE], a_half, f_exp[:, : h * T_CORE]
                    )
                    nc.vector.tensor_mul(
                        gt[:, h * T_CORE :], a_half, f_exp[:, h * T_CORE :]
                    )
                else:
                    nc.vector.tensor_mul(gt[:], a_view, f_exp[:])

                for k in range(sz):
                    i = i0 + k
                    nc.tensor.matmul(
                        ps[:],
                        c_sb[:, k * D : (k + 1) * D],
                        gt[:, k * T_CORE : (k + 1) * T_CORE],
                        start=(i == 0),
                        stop=(i == D - 1),
                    )
                i0 += sz

            o_sb = out_pool.tile([D, T_CORE], F32, tag="o")
            if SPLIT_OUT:
                h = T_CORE // 2
                nc.vector.tensor_copy(o_sb[:, :h], ps[:, :h])
                nc.sync.dma_start(out_t[:, :h], o_sb[:, :h])
                nc.scalar.copy(o_sb[:, h:], ps[:, h:])
                nc.scalar.dma_start(out_t[:, h:], o_sb[:, h:])
            else:
                nc.vector.tensor_copy(o_sb[:], ps[:])
                nc.sync.dma_start(out_t[:, :], o_sb[:])

    nc.compile()
    return nc


def _get_nc():
    global _NC_CACHE
    if _NC_CACHE is None:
        _NC_CACHE = _build()
    return _NC_CACHE


def _prep_in_maps(func_and_arg, cooccurrences):
    fa = np.asarray(func_and_arg, dtype=np.float32).reshape(T_TOTAL, 2 * D)
    c2 = (
        np.ascontiguousarray(
            np.asarray(cooccurrences, dtype=np.float32).transpose(1, 0, 2)
        )
        .reshape(D, D * D)
        .astype(NP_BF16)
    )
    in_maps = []
    for c in range(N_CORES):
        s = fa[c * T_CORE : (c + 1) * T_CORE]  # [512, 256]
        f_tc = np.ascontiguousarray(s[:, :D].T).astype(NP_BF16)  # [128 i, 512 t]
        a_tc = np.ascontiguousarray(s[:, D:].T).astype(NP_BF16)  # [128 j, 512 t]
        in_maps.append({"f_t": f_tc, "a_t": a_tc, "c2": c2})
    return in_maps


def kernel(func_and_arg: np.ndarray, cooccurrences: np.ndarray) -> np.ndarray:
    assert func_and_arg.shape == (4, 1024, 2 * D)
    assert cooccurrences.shape == (D, D, D)

    in_maps = _prep_in_maps(func_and_arg, cooccurrences)
    nc = _get_nc()
    res = run_bass_kernel_spmd(nc, in_maps, core_ids=list(range(N_CORES)))

    # out_t per core: [z=128, t=512] -> [t, z]; concat over cores -> [4096, 128]
    outs = [res.results[c]["out_t"].T for c in range(N_CORES)]
    out = np.concatenate(outs, axis=0).reshape(4, 1024, D).astype(np.float32)
    return out


# revision 19
# speedup vs baseline: 1.3775x; 1.0350x over previous
"""Trainium2 Bass kernel for CoocOpModel.

out[b,s,z] = sum_{i,j} func[b,s,i] * cooc[i,j,z] * arg[b,s,j]
  with func = func_and_arg[..., :128], arg = func_and_arg[..., 128:]

Shapes (hardcoded): func_and_arg [4,1024,256] f32, cooccurrences [128,128,128] f32,
out [4,1024,128] f32.  D = 128, tokens T = 4096.

Strategy: data-parallel over tokens across 8 cores (512 tokens/core);
cooccurrence tensor replicated per core (fp16).

Per-core math, with t = local token index (512), i/j/z in [0,128):
  out_T[z, t] = sum_i  C_i^T @ G_i        (accumulated in one PSUM bank)
  C_i[j, z]   = cooc[i, j, z]             (stationary operand, fp16)
  G_i[j, t]   = arg_T[j, t] * func_T[i, t]  (moving operand, fp16)

i's are processed in groups of GRP=8:
  - one broadcast-DMA materializes f_exp_g[j, (k,t)] = func_T[8g+k, t]
    (replicated across the 128 j-partitions; DRAM-source AP with
    partition-step 0 — SBUF sources reject step-0 partition dims)
  - one DVE tensor-tensor multiply builds G for the whole group, re-reading
    arg_T per k through a free-dim step-0 AP (no materialized a_rep)
  - 8 accumulating matmuls consume it (stationary = per-group cooc tile)
"""

import os
import sys

sys.path.insert(0, "/opt/trn_rl_repo")

import numpy as np
from contextlib import ExitStack

import concourse.bass as bass
import concourse.tile as tile
from concourse import bacc, mybir
from concourse.bass_utils import run_bass_kernel_spmd

BF16 = mybir.dt.float16
F32 = mybir.dt.float32
NP_BF16 = np.float16

N_CORES = 8
D = 128
T_TOTAL = 4096
T_CORE = T_TOTAL // N_CORES  # 512
GRP = 8
N_GRP = D // GRP

# schedule knobs (env-overridable for experiments; defaults = shipped config).
# Small head groups get the pipeline running early; 16-row middle groups use
# 16KB-per-partition DMA descriptors (fewer DGE configs, better fabric
# throughput); small tail groups shorten the serial drain.
FEXP_BUFS = int(os.environ.get("FEXP_BUFS", "6"))
SIZES = (
    [int(x) for x in os.environ["SIZES"].split(",")]
    if os.environ.get("SIZES")
    else [4, 4, 16, 16, 16, 16, 16, 16, 8, 8, 4, 4]
)
SPLIT_OUT = os.environ.get("SPLIT_OUT", "1") == "1"

_NC_CACHE = None


def _build():
    nc = bacc.Bacc("TRN2", target_bir_lowering=False, debug=False, num_devices=N_CORES)

    f_t = nc.dram_tensor("f_t", [D, T_CORE], BF16, kind="ExternalInput").ap()
    a_t = nc.dram_tensor("a_t", [D, T_CORE], BF16, kind="ExternalInput").ap()
    # c2[j, i*128 + z] = cooc[i, j, z]
    c2 = nc.dram_tensor("c2", [D, D * D], BF16, kind="ExternalInput").ap()
    out_t = nc.dram_tensor("out_t", [D, T_CORE], F32, kind="ExternalOutput").ap()

    with tile.TileContext(nc) as tc:
        with ExitStack() as ctx:
            const_pool = ctx.enter_context(tc.tile_pool(name="const", bufs=1))
            fexp_pool = ctx.enter_context(tc.tile_pool(name="fexp", bufs=FEXP_BUFS))
            g_pool = ctx.enter_context(tc.tile_pool(name="g", bufs=3))
            out_pool = ctx.enter_context(tc.tile_pool(name="out", bufs=1))
            psum_pool = ctx.enter_context(
                tc.tile_pool(name="psum", bufs=1, space="PSUM")
            )

            # arg_T in SBUF; the TT re-reads it per k via a free-step-0 AP.
            a_sb = const_pool.tile([D, T_CORE], BF16, tag="a")
            nc.sync.dma_start(a_sb[:], a_t[:, :])
            a_ap = a_sb[:]

            sizes = SIZES
            assert sum(sizes) == D

            ps = psum_pool.tile([D, T_CORE], F32)
            i0 = 0
            for g, sz in enumerate(sizes):
                # f_exp[j, (k, t)] = func_T[i0+k, t], replicated over j.
                f_exp = fexp_pool.tile([D, sz * T_CORE], BF16, tag="fexp")
                if g == 0:
                    half = sz // 2
                    f_src_a = bass.AP(
                        f_t.tensor, i0 * T_CORE, [[0, D], [T_CORE, half], [1, T_CORE]]
                    )
                    f_src_b = bass.AP(
                        f_t.tensor,
                        (i0 + half) * T_CORE,
                        [[0, D], [T_CORE, half], [1, T_CORE]],
                    )
                    nc.scalar.dma_start(f_exp[:, : half * T_CORE], f_src_a)
                    nc.sync.dma_start(f_exp[:, half * T_CORE :], f_src_b)
                else:
                    f_src = bass.AP(
                        f_t.tensor,
                        i0 * T_CORE,
                        [[0, D], [T_CORE, sz], [1, T_CORE]],
                    )
                    eng = nc.sync if g % 2 == 0 else nc.scalar
                    eng.dma_start(f_exp[:], f_src)

                # per-group cooc tile: c_sb[j, (k, z)] = cooc[i0+k, j, z]
                c_sb = const_pool.tile([D, sz * D], BF16, tag=f"c{g}")
                eng = nc.scalar if g % 2 == 0 else nc.sync
                eng.dma_start(c_sb[:], c2[:, i0 * D : (i0 + sz) * D])

                a_view = bass.AP(
                    a_ap.tensor, a_ap.offset, [a_ap.ap[0], [0, sz], [1, T_CORE]]
                )
                gt = g_pool.tile([D, sz * T_CORE], BF16, tag="g")
                if g == len(sizes) - 1 and sz >= 2:
                    # split the last multiply so its first matmuls overlap
                    # the second half — trims the serial tail
                    h = sz // 2
                    a_half = bass.AP(
                        a_ap.tensor, a_ap.offset, [a_ap.ap[0], [0, h], [1, T_CORE]]
                    )
                    nc.vector.tensor_mul(
                        gt[:, : h * T_CORE], a_half, f_exp[:, : h * T_CORE]
                    )
                    nc.vector.tensor_mul(
                        gt[:, h * T_CORE :], a_half, f_exp[:, h * T_CORE :]
                    )
                else:
                    nc.vector.tensor_mul(gt[:], a_view, f_exp[:])

                for k in range(sz):
                    i = i0 + k
                    nc.tensor.matmul(
                        ps[:],
                        c_sb[:, k * D : (k + 1) * D],
                        gt[:, k * T_CORE : (k + 1) * T_CORE],
                        start=(i == 0),
                        stop=(i == D - 1),
                    )
                i0 += sz

            o_sb = out_pool.tile([D, T_CORE], F32, tag="o")
            if SPLIT_OUT:
                h = T_CORE // 2
                nc.vector.tensor_copy(o_sb[:, :h], ps[:, :h])
                nc.sync.dma_start(out_t[:, :h], o_sb[:, :h])
                nc.scalar.copy(o_sb[:, h:], ps[:, h:])
                nc.scalar.dma_start(out_t[:, h:], o_sb[:, h:])
            else:
                nc.vector.tensor_copy(o_sb[:], ps[:])
                nc.sync.dma_start(out_t[:, :], o_sb[:])

    nc.compile()
    return nc


def _get_nc():
    global _NC_CACHE
    if _NC_CACHE is None:
        _NC_CACHE = _build()
    return _NC_CACHE


def _prep_in_maps(func_and_arg, cooccurrences):
    fa = np.asarray(func_and_arg, dtype=np.float32).reshape(T_TOTAL, 2 * D)
    c2 = (
        np.ascontiguousarray(
            np.asarray(cooccurrences, dtype=np.float32).transpose(1, 0, 2)
        )
        .reshape(D, D * D)
        .astype(NP_BF16)
    )
    in_maps = []
    for c in range(N_CORES):
        s = fa[c * T_CORE : (c + 1) * T_CORE]  # [512, 256]
        f_tc = np.ascontiguousarray(s[:, :D].T).astype(NP_BF16)  # [128 i, 512 t]
        a_tc = np.ascontiguousarray(s[:, D:].T).astype(NP_BF16)  # [128 j, 512 t]
        in_maps.append({"f_t": f_tc, "a_t": a_tc, "c2": c2})
    return in_maps


def kernel(func_and_arg: np.ndarray, cooccurrences: np.ndarray) -> np.ndarray:
    assert func_and_arg.shape == (4, 1024, 2 * D)
    assert cooccurrences.shape == (D, D, D)

    in_maps = _prep_in_maps(func_and_arg, cooccurrences)
    nc = _get_nc()
    res = run_bass_kernel_spmd(nc, in_maps, core_ids=list(range(N_CORES)))

    # out_t per core: [z=128, t=512] -> [t, z]; concat over cores -> [4096, 128]
    outs = [res.results[c]["out_t"].T for c in range(N_CORES)]
    out = np.concatenate(outs, axis=0).reshape(4, 1024, D).astype(np.float32)
    return out
